# revision 14
# baseline (speedup 1.0000x reference)
"""BiMamba Trainium2 kernel — 8-core SPMD.

Sharding: core = b*4 + dir*2 + nh  (b: batch, dir: fwd/rev, nh: state half).
Each core runs the full mamba pipeline for its (b, dir) on all 768 inner
channels but only its 8 of 16 SSM states, pushes its partial through its
direction's half of the final 1x1 conv, then one ReduceScatter(add) per
batch group of 4 cores both sums the partials and hands each core 192
(permuted) channels = 96 GLU `a` channels + their 96 `b` partners.
GLU + GroupNorm finish locally (tiny AllReduce for the GN stats).

Layouts are channel-major [channel partitions x time free] throughout; the
selective scan runs as hardware tensor_tensor_scan along the free (time)
axis with fp32 carried state across time chunks.
"""
import os
import numpy as np
import ml_dtypes

import concourse.bass as bass
import concourse.bacc as bacc_mod
import concourse.mybir as mybir
import concourse.tile as tile
from concourse.bass_utils import run_bass_kernel_spmd

F32 = mybir.dt.float32
BF16 = mybir.dt.bfloat16
AF = mybir.ActivationFunctionType
OP = mybir.AluOpType

D_MODEL = 384
D_INNER = 768
D_STATE = 16
D_CONV = 4
DT_RANK = 24
B = 2
L = 4096
T = 512                 # time chunk
NCH = L // T
NH = 8                  # states per core
NM = 40                 # xproj out rows: 24 dt + 8 B + 8 C
RG = [[0, 1, 2, 3], [4, 5, 6, 7]]   # batch groups
GN_N = float(D_MODEL * L)

bf = ml_dtypes.bfloat16


def build_program():
    nc = bacc_mod.Bacc(num_devices=8)

    # ---------------- DRAM I/O (per-core values supplied via in_maps) -------
    x_in = nc.dram_tensor("x_bc", [128, 3, L + D_CONV - 1], BF16, kind="ExternalInput")
    w_in = nc.dram_tensor("w_in", [128, 3, 2 * D_INNER], BF16, kind="ExternalInput")
    w_xp = nc.dram_tensor("w_xp", [128, 6, NM], BF16, kind="ExternalInput")
    w_dt = nc.dram_tensor("w_dt", [DT_RANK, D_INNER], BF16, kind="ExternalInput")
    w_out = nc.dram_tensor("w_out", [128, 6, D_MODEL], BF16, kind="ExternalInput")
    w_zc = nc.dram_tensor("w_zc", [128, 3, 2 * D_MODEL], BF16, kind="ExternalInput")
    conv_w = nc.dram_tensor("conv_w", [128, 6, D_CONV], F32, kind="ExternalInput")
    conv_b = nc.dram_tensor("conv_b", [128, 6], F32, kind="ExternalInput")
    dt_b = nc.dram_tensor("dt_b", [128, 6], F32, kind="ExternalInput")
    d_skip = nc.dram_tensor("d_skip", [128, 6], F32, kind="ExternalInput")
    a_sc = nc.dram_tensor("a_sc", [128, 6, NH], F32, kind="ExternalInput")
    cb_a = nc.dram_tensor("cb_a", [96, 1], F32, kind="ExternalInput")
    cb_b = nc.dram_tensor("cb_b", [96, 1], F32, kind="ExternalInput")
    gnw = nc.dram_tensor("gnw", [96, 1], F32, kind="ExternalInput")
    gnb = nc.dram_tensor("gnb", [96, 1], F32, kind="ExternalInput")
    y_out = nc.dram_tensor("y_out", [96, L], F32, kind="ExternalOutput")

    # internal DRAM
    bc_stage = nc.dram_tensor("bc_stage", [2 * NH, L], BF16)
    z_part = nc.dram_tensor("z_part", [D_INNER, L], BF16)
    z_red = nc.dram_tensor("z_red", [192, L], BF16)
    gn_in = nc.dram_tensor("gn_in", [1, 2], F32)
    gn_out = nc.dram_tensor("gn_out", [1, 2], F32)
    mr_dram = nc.dram_tensor("mr_dram", [1, 2], F32)

    ident_dram = nc.inline_tensor(np.eye(128, dtype=bf), name="ident")

    with tile.TileContext(nc) as tc:
        _body(tc, nc, x_in, w_in, w_xp, w_dt, w_out, w_zc, conv_w, conv_b,
              dt_b, d_skip, a_sc, cb_a, cb_b, gnw, gnb, y_out,
              bc_stage, z_part, z_red, gn_in, gn_out, mr_dram, ident_dram)
    if not nc.is_finalized():
        nc.finalize()   # Bacc: runs compile passes (incl. sync-wait splitting)
    return nc


def _body(tc, nc, x_in, w_in, w_xp, w_dt, w_out, w_zc, conv_w, conv_b,
          dt_b, d_skip, a_sc, cb_a, cb_b, gnw, gnb, y_out,
          bc_stage, z_part, z_red, gn_in, gn_out, mr_dram, ident_dram):
    from contextlib import ExitStack

    with ExitStack() as ctx:
        # ------------------------- persistent tiles -------------------------
        singles = ctx.enter_context(tc.tile_pool(name="singles", bufs=1))
        sb_win = singles.tile([128, 3, 2 * D_INNER], BF16)
        nc.sync.dma_start(out=sb_win, in_=w_in[:])
        sb_wxp = singles.tile([128, 6, NM], BF16)
        nc.sync.dma_start(out=sb_wxp, in_=w_xp[:])
        sb_wdt = singles.tile([DT_RANK, D_INNER], BF16)
        nc.sync.dma_start(out=sb_wdt, in_=w_dt[:])
        sb_wout = singles.tile([128, 6, D_MODEL], BF16)
        nc.sync.dma_start(out=sb_wout, in_=w_out[:])
        sb_wzc = singles.tile([128, 3, 2 * D_MODEL], BF16)
        nc.sync.dma_start(out=sb_wzc, in_=w_zc[:])
        sb_cw = singles.tile([128, 6, D_CONV], F32)
        nc.sync.dma_start(out=sb_cw, in_=conv_w[:])
        sb_cb = singles.tile([128, 6], F32)
        nc.sync.dma_start(out=sb_cb, in_=conv_b[:])
        sb_dtb = singles.tile([128, 6], F32)
        nc.sync.dma_start(out=sb_dtb, in_=dt_b[:])
        sb_dsk = singles.tile([128, 6], F32)
        nc.sync.dma_start(out=sb_dsk, in_=d_skip[:])
        sb_asc = singles.tile([128, 6, NH], F32)
        nc.sync.dma_start(out=sb_asc, in_=a_sc[:])
        sb_id = singles.tile([128, 128], BF16)
        nc.sync.dma_start(out=sb_id, in_=ident_dram[:])

        sb_carry = singles.tile([128, 6, NH], F32)
        nc.vector.memset(sb_carry, 0.0)

        # ------------------------- pools -------------------------
        psum_mm = ctx.enter_context(tc.tile_pool(name="psum_mm", bufs=3, space="PSUM"))
        psum_y = ctx.enter_context(tc.tile_pool(name="psum_y", bufs=3, space="PSUM"))

        p_x = ctx.enter_context(tc.tile_pool(name="p_x", bufs=2))
        p_xi = ctx.enter_context(tc.tile_pool(name="p_xi", bufs=2))
        p_sz = ctx.enter_context(tc.tile_pool(name="p_sz", bufs=2))
        p_xc = ctx.enter_context(tc.tile_pool(name="p_xc", bufs=2))
        p_xdbl = ctx.enter_context(tc.tile_pool(name="p_xdbl", bufs=2))
        p_cpre = ctx.enter_context(tc.tile_pool(name="p_cpre", bufs=2))
        p_dl = ctx.enter_context(tc.tile_pool(name="p_dl", bufs=2))
        p_dx = ctx.enter_context(tc.tile_pool(name="p_dx", bufs=2))
        p_bc = ctx.enter_context(tc.tile_pool(name="p_bc", bufs=4))
        p_da = ctx.enter_context(tc.tile_pool(name="p_da", bufs=3))
        p_u = ctx.enter_context(tc.tile_pool(name="p_u", bufs=3))
        p_h = ctx.enter_context(tc.tile_pool(name="p_h", bufs=4))
        p_q = ctx.enter_context(tc.tile_pool(name="p_q", bufs=3))
        p_t1 = ctx.enter_context(tc.tile_pool(name="p_t1", bufs=2))
        p_gt = ctx.enter_context(tc.tile_pool(name="p_gt", bufs=2))
        p_ydm = ctx.enter_context(tc.tile_pool(name="p_ydm", bufs=2))
        p_zc = ctx.enter_context(tc.tile_pool(name="p_zc", bufs=3))
        p_tmp = ctx.enter_context(tc.tile_pool(name="p_tmp", bufs=3))

        HW = D_CONV - 1      # halo width
        for c in range(NCH):
            sl = slice(c * T, (c + 1) * T)

            # ---- load x chunk with leading halo (x_in is host-padded) ----
            sb_x = p_x.tile([128, 3, T + HW], BF16, tag="x")
            nc.sync.dma_start(out=sb_x, in_=x_in[:, :, c * T:c * T + T + HW])

            # ---- in_proj: 12 M-tiles (6 xi, 6 z) + tiny halo matmuls ----
            sb_xi = p_xi.tile([128, 6, T + HW], F32, tag="xi")
            sb_sz = p_sz.tile([128, 6, T], BF16, tag="sz")
            for mt in range(6):
                psh = psum_mm.tile([128, HW], F32, tag="mmh", bufs=1)
                for kt in range(3):
                    nc.tensor.matmul(psh, sb_win[:, kt, mt * 128:(mt + 1) * 128],
                                     sb_x[:, kt, 0:HW], start=(kt == 0), stop=(kt == 2))
                nc.scalar.copy(out=sb_xi[:, mt, 0:HW], in_=psh)
            for mt in range(12):
                ps = psum_mm.tile([128, T], F32, tag="mm")
                for kt in range(3):
                    nc.tensor.matmul(ps, sb_win[:, kt, mt * 128:(mt + 1) * 128],
                                     sb_x[:, kt, HW:HW + T], start=(kt == 0), stop=(kt == 2))
                if mt < 6:
                    nc.scalar.copy(out=sb_xi[:, mt, HW:], in_=ps)
                else:
                    zt = mt - 6
                    # silu(z) = z * sigmoid(z)
                    sg = p_tmp.tile([128, T], BF16, tag="sg")
                    nc.scalar.activation(out=sg, in_=ps, func=AF.Sigmoid)
                    nc.vector.tensor_tensor(out=sb_sz[:, zt, :], in0=ps, in1=sg,
                                            op=OP.mult)

            # ---- causal depthwise conv + bias + silu ----
            sb_xc = p_xc.tile([128, 6, T], BF16, tag="xc")
            for dt in range(6):
                cp = p_cpre.tile([128, T], F32, tag="cpre")
                nc.vector.tensor_scalar_mul(out=cp, in0=sb_xi[:, dt, 0:T],
                                            scalar1=sb_cw[:, dt, 0:1])
                for k in range(1, D_CONV):
                    nc.vector.scalar_tensor_tensor(
                        out=cp, in0=sb_xi[:, dt, k:k + T],
                        scalar=sb_cw[:, dt, k:k + 1], in1=cp,
                        op0=OP.mult, op1=OP.add)
                # xc = (cp+b) * sigmoid(cp+b)
                sgc = p_tmp.tile([128, T], BF16, tag="sgc")
                nc.scalar.activation(out=sgc, in_=cp, func=AF.Sigmoid,
                                     bias=sb_cb[:, dt:dt + 1], scale=1.0)
                nc.vector.scalar_tensor_tensor(
                    out=sb_xc[:, dt, :], in0=cp, scalar=sb_cb[:, dt:dt + 1],
                    in1=sgc, op0=OP.add, op1=OP.mult)

            # ---- xproj -> xdbl [40, T] ----
            psx = psum_mm.tile([NM, T], F32, tag="xp", bufs=1)
            for kt in range(6):
                nc.tensor.matmul(psx, sb_wxp[:, kt, :], sb_xc[:, kt, :],
                                 start=(kt == 0), stop=(kt == 5))
            sb_xdbl = p_xdbl.tile([NM, T], BF16, tag="xdbl")
            nc.scalar.copy(out=sb_xdbl, in_=psx)
            # stage B/C rows for broadcast
            nc.sync.dma_start(out=bc_stage[:, sl], in_=sb_xdbl[DT_RANK:NM, :])

            # ---- dt-proj -> delta (softplus via exp/ln) ----
            sb_dl = p_dl.tile([128, 6, T], F32, tag="dl")
            for mt in range(6):
                ps = psum_mm.tile([128, T], F32, tag="mm")
                nc.tensor.matmul(ps, sb_wdt[:, mt * 128:(mt + 1) * 128],
                                 sb_xdbl[0:DT_RANK, :], start=True, stop=True)
                ex = p_tmp.tile([128, T], F32, tag="ex")
                nc.scalar.activation(out=ex, in_=ps, func=AF.Exp,
                                     bias=sb_dtb[:, mt:mt + 1], scale=1.0)
                nc.scalar.activation(out=sb_dl[:, mt, :], in_=ex, func=AF.Ln,
                                     bias=1.0, scale=1.0)

            # ---- delta * xc ----
            sb_dx = p_dx.tile([128, 6, T], BF16, tag="dx")
            for dt in range(6):
                nc.vector.tensor_tensor(out=sb_dx[:, dt, :], in0=sb_dl[:, dt, :],
                                        in1=sb_xc[:, dt, :], op=OP.mult)

            # ---- scan waves ----
            sb_gt = p_gt.tile([128, 6, T], BF16, tag="gt")
            bc_cache = {}
            for wv in range(2):
                pys = [psum_y.tile([128, T], F32, tag="py", name=f"py_{c}_{wv}_{j}")
                       for j in range(3)]
                for n in range(NH):
                    if wv == 0:
                        bb = p_bc.tile([128, T], BF16, tag="bb%d" % (n % 2))
                        nc.sync.dma_start(
                            out=bb, in_=bc_stage[n, sl].partition_broadcast(128))
                        cc = p_bc.tile([128, T], BF16, tag="cc%d" % (n % 2))
                        nc.sync.dma_start(
                            out=cc, in_=bc_stage[NH + n, sl].partition_broadcast(128))
                        bb_n, cc_n = bb, cc
                        bc_cache[n] = (bb, cc)
                    else:
                        bb_n, cc_n = bc_cache[n]
                    for j in range(3):
                        dt = wv * 3 + j
                        da = p_da.tile([128, T], F32, tag="da")
                        nc.scalar.activation(out=da, in_=sb_dl[:, dt, :],
                                             func=AF.Exp,
                                             scale=sb_asc[:, dt, n:n + 1])
                        u = p_u.tile([128, T], BF16, tag="u")
                        nc.vector.tensor_tensor(out=u, in0=sb_dx[:, dt, :],
                                                in1=bb_n, op=OP.mult)
                        h = p_h.tile([128, T], BF16, tag="h")
                        nc.vector.tensor_tensor_scan(
                            out=h, data0=da, data1=u,
                            initial=sb_carry[:, dt, n:n + 1],
                            op0=OP.mult, op1=OP.add)
                        nc.gpsimd.tensor_copy(out=sb_carry[:, dt, n:n + 1],
                                              in_=h[:, T - 1:T])
                        q = p_q.tile([128, T], BF16, tag="q")
                        nc.vector.tensor_tensor(out=q, in0=h, in1=cc_n, op=OP.mult)
                        nc.tensor.matmul(pys[j], sb_id, q,
                                         start=(n == 0), stop=(n == NH - 1))
                # skip + gate
                for j in range(3):
                    dt = wv * 3 + j
                    t1 = p_t1.tile([128, T], F32, tag="t1")
                    nc.vector.scalar_tensor_tensor(
                        out=t1, in0=sb_xc[:, dt, :], scalar=sb_dsk[:, dt:dt + 1],
                        in1=pys[j], op0=OP.mult, op1=OP.add)
                    nc.vector.tensor_tensor(out=sb_gt[:, dt, :], in0=t1,
                                            in1=sb_sz[:, dt, :], op=OP.mult)

            # ---- out_proj ----
            sb_ydm = p_ydm.tile([128, 3, T], BF16, tag="ydm")
            for mt in range(3):
                ps = psum_mm.tile([128, T], F32, tag="mm")
                for kt in range(6):
                    nc.tensor.matmul(ps, sb_wout[:, kt, mt * 128:(mt + 1) * 128],
                                     sb_gt[:, kt, :], start=(kt == 0), stop=(kt == 5))
                nc.vector.tensor_copy(out=sb_ydm[:, mt, :], in_=ps)

            # ---- z-conv partial (permuted output channels) ----
            for mt in range(6):
                ps = psum_mm.tile([128, T], F32, tag="mm")
                for kt in range(3):
                    nc.tensor.matmul(ps, sb_wzc[:, kt, mt * 128:(mt + 1) * 128],
                                     sb_ydm[:, kt, :], start=(kt == 0), stop=(kt == 2))
                zc = p_zc.tile([128, T], BF16, tag="zc")
                nc.vector.tensor_copy(out=zc, in_=ps)
                nc.sync.dma_start(out=z_part[mt * 128:(mt + 1) * 128, sl], in_=zc)

    # ---------------- ReduceScatter + GLU + GroupNorm ----------------
    nc.gpsimd.collective_compute(
        "ReduceScatter", OP.add, replica_groups=RG,
        ins=[z_part[:]], outs=[z_red[:]])

    with ExitStack() as ctx:
        fin = ctx.enter_context(tc.tile_pool(name="fin", bufs=1))
        psf = ctx.enter_context(tc.tile_pool(name="psf", bufs=2, space="PSUM"))

        sb_cba = fin.tile([96, 1], F32)
        nc.sync.dma_start(out=sb_cba, in_=cb_a[:])
        sb_cbb = fin.tile([96, 1], F32)
        nc.sync.dma_start(out=sb_cbb, in_=cb_b[:])
        sb_gnw = fin.tile([96, 1], F32)
        nc.sync.dma_start(out=sb_gnw, in_=gnw[:])
        sb_gnb = fin.tile([96, 1], F32)
        nc.sync.dma_start(out=sb_gnb, in_=gnb[:])

        sb_a = fin.tile([96, L], BF16)
        nc.sync.dma_start(out=sb_a, in_=z_red[0:96, :])
        sb_b = fin.tile([96, L], BF16)
        nc.sync.dma_start(out=sb_b, in_=z_red[96:192, :])

        sg = fin.tile([96, L], BF16)
        nc.scalar.activation(out=sg, in_=sb_b, func=AF.Sigmoid,
                             bias=sb_cbb[:, 0:1], scale=1.0)
        yglu = fin.tile([96, L], F32)
        nc.vector.scalar_tensor_tensor(out=yglu, in0=sb_a, scalar=sb_cba[:, 0:1],
                                       in1=sg, op0=OP.add, op1=OP.mult)

        # GN stats: per-partition sum / sumsq, then partition-reduce via PE
        scr = fin.tile([96, L], BF16)
        ssum = fin.tile([96, 1], F32)
        nc.scalar.activation(out=scr, in_=yglu, func=AF.Copy, accum_out=ssum)
        ssq = fin.tile([96, 1], F32)
        nc.scalar.activation(out=scr, in_=yglu, func=AF.Square, accum_out=ssq)
        stats = fin.tile([96, 2], F32)
        nc.gpsimd.tensor_copy(out=stats[:, 0:1], in_=ssum)
        nc.gpsimd.tensor_copy(out=stats[:, 1:2], in_=ssq)
        ones = fin.tile([96, 1], F32)
        nc.vector.memset(ones, 1.0)
        pss = psf.tile([1, 2], F32, tag="pss")
        nc.tensor.matmul(pss, ones, stats, start=True, stop=True)
        s_loc = fin.tile([1, 2], F32)
        nc.vector.tensor_copy(out=s_loc, in_=pss)
        nc.sync.dma_start(out=gn_in[:], in_=s_loc)
        nc.gpsimd.collective_compute(
            "AllReduce", OP.add, replica_groups=RG,
            ins=[gn_in[:]], outs=[gn_out[:]])
        s_glob = fin.tile([1, 2], F32)
        nc.sync.dma_start(out=s_glob, in_=gn_out[:])

        mu = fin.tile([1, 1], F32)
        nc.scalar.mul(out=mu, in_=s_glob[:, 0:1], mul=1.0 / GN_N)
        ms = fin.tile([1, 1], F32)
        nc.scalar.mul(out=ms, in_=s_glob[:, 1:2], mul=1.0 / GN_N)
        mu2 = fin.tile([1, 1], F32)
        nc.scalar.activation(out=mu2, in_=mu, func=AF.Square)
        var = fin.tile([1, 1], F32)
        nc.vector.tensor_tensor(out=var, in0=ms, in1=mu2, op=OP.subtract)
        eps_sb = fin.tile([1, 1], F32)
        nc.vector.memset(eps_sb, 1e-5)
        std = fin.tile([1, 1], F32)
        nc.scalar.activation(out=std, in_=var, func=AF.Sqrt,
                             bias=eps_sb[:, 0:1], scale=1.0)
        rstd = fin.tile([1, 1], F32)
        nc.vector.reciprocal(out=rstd, in_=std)
        mr = fin.tile([1, 2], F32)
        nc.gpsimd.tensor_copy(out=mr[:, 0:1], in_=mu)
        nc.gpsimd.tensor_copy(out=mr[:, 1:2], in_=rstd)
        nc.sync.dma_start(out=mr_dram[:], in_=mr)
        mr96 = fin.tile([96, 2], F32)
        nc.sync.dma_start(out=mr96, in_=mr_dram[0, :].partition_broadcast(96))

        scale = fin.tile([96, 1], F32)
        nc.vector.tensor_tensor(out=scale, in0=sb_gnw, in1=mr96[:, 1:2],
                                op=OP.mult)
        y1 = fin.tile([96, L], F32)
        nc.vector.tensor_scalar(out=y1, in0=yglu, scalar1=mr96[:, 0:1],
                                scalar2=scale, op0=OP.subtract, op1=OP.mult)
        y2 = fin.tile([96, L], F32)
        nc.vector.tensor_scalar_add(out=y2, in0=y1, scalar1=sb_gnb[:, 0:1])
        nc.sync.dma_start(out=y_out[:], in_=y2)


_bc_cache = {}


# ======================= host side =======================

def _tiles_pmajor(w, p=128):
    """[R, C] -> [p, R//p, C] partition-major tiles."""
    r, cdim = w.shape
    return np.ascontiguousarray(
        w.reshape(r // p, p, cdim).transpose(1, 0, 2))


def _vec6(v):
    return np.ascontiguousarray(v.reshape(6, 128).T)


_PROG = None


def _get_prog():
    global _PROG
    if _PROG is None:
        _PROG = build_program()
    return _PROG


def make_in_maps(inputs):
    x = np.asarray(inputs['x'], np.float32)
    c_w = np.asarray(inputs['c_w'], np.float32)[:, :, 0]
    c_b = np.asarray(inputs['c_b'], np.float32)
    gn_w = np.asarray(inputs['gn_w'], np.float32)
    gn_b = np.asarray(inputs['gn_b'], np.float32)

    perm = []
    for r in range(4):
        perm += list(range(r * 96, (r + 1) * 96))
        perm += list(range(D_MODEL + r * 96, D_MODEL + (r + 1) * 96))
    perm = np.array(perm)
    c_w_p = c_w[perm]
    c_b_p = c_b[perm]

    in_maps = []
    for core in range(8):
        b, rem = divmod(core, 4)
        dirn, nh = divmod(rem, 2)
        rank = rem
        pref = 'f_' if dirn == 0 else 'b_'
        g = lambda k: np.asarray(inputs[pref + k], np.float32)

        x_bc = x[b] if dirn == 0 else x[b, :, ::-1]
        in_w = g('in_w')                    # [1536, 384]
        xproj_w = g('xproj_w')              # [56, 768]
        rows = np.concatenate([
            xproj_w[:DT_RANK],
            xproj_w[DT_RANK + nh * NH: DT_RANK + (nh + 1) * NH],
            xproj_w[DT_RANK + D_STATE + nh * NH: DT_RANK + D_STATE + (nh + 1) * NH],
        ], 0)                               # [40, 768]
        A = -np.exp(g('A_log'))             # [768, 16]
        Dp = g('D') if nh == 0 else np.zeros(D_INNER, np.float32)
        wc_slice = c_w_p[:, dirn * D_MODEL:(dirn + 1) * D_MODEL]  # [768, 384]

        m = {
            'x_bc': _tiles_pmajor(np.concatenate(
                [np.zeros((D_MODEL, D_CONV - 1), np.float32),
                 np.ascontiguousarray(x_bc)], axis=1)).astype(bf),
            'w_in': _tiles_pmajor(in_w.T).astype(bf),            # [384,1536] -> tiles
            'w_xp': _tiles_pmajor(rows.T).astype(bf),            # [768,40]
            'w_dt': np.ascontiguousarray(g('dt_w').T).astype(bf),  # [24,768]
            'w_out': _tiles_pmajor(g('out_w').T).astype(bf),     # [768,384]
            'w_zc': _tiles_pmajor(np.ascontiguousarray(wc_slice.T)).astype(bf),
            'conv_w': np.ascontiguousarray(
                g('conv_w')[:, 0, :].reshape(6, 128, D_CONV).transpose(1, 0, 2)),
            'conv_b': _vec6(g('conv_b')),
            'dt_b': _vec6(g('dt_b')),
            'd_skip': _vec6(Dp),
            'a_sc': np.ascontiguousarray(
                A[:, nh * NH:(nh + 1) * NH].reshape(6, 128, NH).transpose(1, 0, 2)),
            'cb_a': np.ascontiguousarray(
                c_b_p[rank * 192: rank * 192 + 96].reshape(96, 1)),
            'cb_b': np.ascontiguousarray(
                c_b_p[rank * 192 + 96:(rank + 1) * 192].reshape(96, 1)),
            'gnw': np.ascontiguousarray(
                gn_w[rank * 96:(rank + 1) * 96].reshape(96, 1)),
            'gnb': np.ascontiguousarray(
                gn_b[rank * 96:(rank + 1) * 96].reshape(96, 1)),
        }
        in_maps.append(m)
    return in_maps


def kernel(**inputs):
    nc = _get_prog()
    in_maps = make_in_maps(inputs)
    res = run_bass_kernel_spmd(nc, in_maps, list(range(8)))
    outs = res.results
    out = np.zeros((B, D_MODEL, L), np.float32)
    for core in range(8):
        b, rank = divmod(core, 4)
        out[b, rank * 96:(rank + 1) * 96, :] = outs[core]['y_out']
    return out


if __name__ == "__main__":
    import reference as ref
    inputs = {k: np.asarray(v) for k, v in ref.setup_inputs().items()}
    got = kernel(**inputs)
    exp = np.asarray(ref.reference(**inputs))
    rel = np.linalg.norm(got - exp) / np.linalg.norm(exp)
    print("rel fro err:", rel)
